# revision 13
# baseline (speedup 1.0000x reference)
"""Trainium2 Bass kernel for gated multi-head attention + residual + LayerNorm.

Problem (nn_CNP_5669356834854):
    B=2, L=2048, D=1024, H=16, DK=DV=64
    Q = q@wq.T+bq; K = k@wk.T+bk; V = v@wv.T+bv   (per-head split)
    attn = softmax((Q K^T / sqrt(DK)) * k_gate  [masked])
    out = LayerNorm(attn @ V @ wo.T + bo + q)

Sharding: 8 cores = (batch b in {0,1}) x (head-group hg in {0..3}, 4 heads each).
L1 computes UNNORMALIZED per-head attention outputs O^T plus softmax
denominators (ones-augmented V trick).  L2 shards (batch, 512-row chunk):
normalizes O (one bf16 2x-mode multiply against host-expanded reciprocal
denominators), then output projection + residual + LayerNorm.

L1 performance structure:
  - dual DMA streams: x_k/x_v/gate-slabs on the Sync HWDGE queue,
    weights/x_q/outputs on the GpSimd SWDGE queue (idle engine) so input
    loads and gate streaming overlap.
  - the gate arrives as host-packed contiguous slabs (one 512KB DMA per
    (pr,half,lkt)) for near-peak HBM rate.
  - per-head [128,1024] S-tiles in a 2-slot PSUM ring; gate-multiply on DVE
    streams back-to-back; exp runs on ACT over paired tiles (FD=4096).
  - one global software-pipelined tick stream across all 4 (pr,half)
    blocks: front(g) = S-matmuls + gate-mul + exp, back(g-PIPE) = O-matmul
    accumulation.  Cross-block pipelining removes block-boundary bubbles.
  - every OFFLOAD_EVERYth hp-tile takes the ACT-copy + GpSimd-multiply path
    instead of the DVE path, balancing DVE vs ACT vs GpSimd.
  - a few warm-up matmuls at t=0 get the PE HAM clock to 2.4 GHz before the
    projections start.
"""

import numpy as np
import ml_dtypes

import concourse.bacc as bacc
import concourse.tile as tile
from concourse import mybir
from concourse.bass_utils import run_bass_kernel_spmd

B, L, D, H, DK, DV = 2, 2048, 1024, 16, 64, 64
EPS = 1e-5
NCORE = 8
HPC = 4  # heads per core
NKC = D // 128  # 8 contraction chunks
NLKT = L // 128  # 16 lk tiles
CH = 512  # L2 row-chunk per core
MPC = HPC * DK  # 256 projected rows per core
HF = L // 2  # 1024, lq per (pr, half) block
EXP_BIAS = -20.0

F32 = mybir.dt.float32
BF16 = mybir.dt.bfloat16
NPBF16 = ml_dtypes.bfloat16
AF = mybir.ActivationFunctionType

PIPE = 6  # O-matmul lag in global ticks
OFFLOAD_EVERY = 8  # every Nth hp-tile -> ACT-copy + GpSimd-mul path (0=off)
N_WARM = 12  # warm-up matmuls


def _bf(x):
    return np.ascontiguousarray(x).astype(NPBF16)


def _kc_layout(a):
    """[D, N] -> [128, NKC, N] with row r = kc*128+p  ->  [p, kc, :]."""
    d, n = a.shape
    assert d == NKC * 128
    return np.ascontiguousarray(a.reshape(NKC, 128, n).transpose(1, 0, 2))


def build_l1(masked: bool, use_bq: bool, use_bk: bool, use_bv: bool):
    nc = bacc.Bacc("TRN2", target_bir_lowering=False)

    qT = nc.declare_dram_parameter("qT", [128, NKC, L], BF16, isOutput=False)
    kT = nc.declare_dram_parameter("kT", [128, NKC, L], BF16, isOutput=False)
    vT = nc.declare_dram_parameter("vT", [128, NKC, L], BF16, isOutput=False)
    wqT = nc.declare_dram_parameter("wqT", [128, NKC, MPC], BF16, isOutput=False)
    wkT = nc.declare_dram_parameter("wkT", [128, NKC, MPC], BF16, isOutput=False)
    wvT = nc.declare_dram_parameter("wvT", [128, NKC, MPC], BF16, isOutput=False)
    # host-packed gate: gPK[pr, half, lkt, p, hp*1024 + c*512 + i]
    gPK = nc.declare_dram_parameter(
        "gPK", [2, 2, NLKT, 128, 2 * HF], BF16, isOutput=False
    )
    if use_bq:
        bqP = nc.declare_dram_parameter("bqP", [128, 2], F32, isOutput=False)
    if use_bk:
        bkP = nc.declare_dram_parameter("bkP", [128, 2], F32, isOutput=False)
    if use_bv:
        bvR = nc.declare_dram_parameter("bvR", [1, MPC], F32, isOutput=False)
    if masked:
        mbT = nc.declare_dram_parameter("mbT", [L, L], BF16, isOutput=False)
    # unnormalized O (rows 0:64 per hp) + denominator (row 64)
    oU = nc.declare_dram_parameter("oU", [2, 2, 65, 2, HF], BF16, isOutput=True)

    BLOCKS = [(0, 0), (0, 1), (1, 0), (1, 1)]

    with tile.TileContext(nc) as tc:
        with (
            tc.tile_pool(name="xs", bufs=4) as xs,
            tc.tile_pool(name="ws", bufs=1) as ws,
            tc.tile_pool(name="qk", bufs=1) as qk,
            tc.tile_pool(name="gp", bufs=5) as gp,
            tc.tile_pool(name="tp", bufs=2) as tp,
            tc.tile_pool(name="pp", bufs=4) as pp,
            tc.tile_pool(name="sb8", bufs=2) as sb8,
            tc.tile_pool(name="op", bufs=2) as opl,
            tc.tile_pool(name="ps_s", bufs=2, space="PSUM") as ps_s,
            tc.tile_pool(name="ps_o", bufs=2, space="PSUM") as ps_o,
        ):
            # ---- weight / input DMAs on the GpSimd (SWDGE) stream ----
            wq_sb = ws.tile([128, NKC, MPC], BF16, tag="wq")
            nc.gpsimd.dma_start(out=wq_sb, in_=wqT[:, :, :])
            wk_sb = ws.tile([128, NKC, MPC], BF16, tag="wk")
            nc.gpsimd.dma_start(out=wk_sb, in_=wkT[:, :, :])
            wv_sb = ws.tile([128, NKC, MPC], BF16, tag="wv")
            nc.gpsimd.dma_start(out=wv_sb, in_=wvT[:, :, :])

            # ---- x loads: column-halves [128, NKC, 1024] ----
            # sync stream: x_k halves then x_v halves (gates follow)
            x_k = {}
            for hh in range(2):
                t = xs.tile([128, NKC, HF], BF16, tag="x", name=f"xk{hh}")
                nc.sync.dma_start(out=t, in_=kT[:, :, hh * HF : (hh + 1) * HF])
                x_k[hh] = t
            x_v = {}
            for hh in range(2):
                t = xs.tile([128, NKC, HF], BF16, tag="x", name=f"xv{hh}")
                nc.sync.dma_start(out=t, in_=vT[:, :, hh * HF : (hh + 1) * HF])
                x_v[hh] = t
            # gpsimd stream: x_q halves
            x_q = {}
            for hh in range(2):
                t = xs.tile([128, NKC, HF], BF16, tag="x", name=f"xq{hh}")
                nc.gpsimd.dma_start(out=t, in_=qT[:, :, hh * HF : (hh + 1) * HF])
                x_q[hh] = t

            QT = qk.tile([128, 2, L], BF16, tag="qt")
            KT = qk.tile([128, 2, L], BF16, tag="kt")
            Vaug = qk.tile([128, NLKT, HPC, 128], BF16, tag="va")
            nc.vector.memset(Vaug[:, :, :, 64:128], 1.0)
            ebias = ws.tile([128, 1], F32, tag="eb")
            nc.vector.memset(ebias, EXP_BIAS)

            bias_tiles = {}
            if use_bq:
                bq_sb = ws.tile([128, 2], F32, tag="bq")
                nc.sync.dma_start(out=bq_sb, in_=bqP[:, :])
                bias_tiles["q"] = bq_sb
            if use_bk:
                bk_sb = ws.tile([128, 2], F32, tag="bk")
                nc.sync.dma_start(out=bk_sb, in_=bkP[:, :])
                bias_tiles["k"] = bk_sb
            if use_bv:
                bv_sb = ws.tile([128, MPC], F32, tag="bv")
                nc.sync.dma_start(out=bv_sb, in_=bvR.ap().to_broadcast([128, MPC]))
                bias_tiles["v"] = bv_sb

            # ---- PE warm-up: matmuls on the weights, result discarded ----
            warm = ps_s.tile([128, 2 * 512], F32, tag="s", name="warm")
            for i in range(N_WARM):
                nc.tensor.matmul(
                    warm[:, 0:MPC],
                    lhsT=wq_sb[:, 0, 0:128],
                    rhs=wq_sb[:, 0, :],
                    start=True,
                    stop=True,
                    skip_group_check=True,
                )

            # ---- projections ----
            def emit_qk_proj(name, xh, w_sb, dst, mt, lqh):
                ps = ps_s.tile(
                    [128, 2 * 512], F32, tag="s", name=f"pj_{name}{mt}{lqh}"
                )
                for c in range(2):
                    for kc in range(NKC):
                        nc.tensor.matmul(
                            ps[:, c * 512 : (c + 1) * 512],
                            lhsT=w_sb[:, kc, mt * 128 : (mt + 1) * 128],
                            rhs=xh[lqh][:, kc, c * 512 : (c + 1) * 512],
                            start=(kc == 0),
                            stop=(kc == NKC - 1),
                        )
                if name in bias_tiles:
                    nc.vector.tensor_scalar_add(
                        out=dst[:, mt, lqh * HF : (lqh + 1) * HF],
                        in0=ps,
                        scalar1=bias_tiles[name][:, mt : mt + 1],
                    )
                else:
                    nc.scalar.copy(
                        out=dst[:, mt, lqh * HF : (lqh + 1) * HF], in_=ps
                    )

            def emit_v_lkt(lkt):
                hh = lkt // 8
                col = (lkt % 8) * 128
                ps = ps_o.tile([128, MPC], F32, tag="o", name="pj_v")
                for kc in range(NKC):
                    nc.tensor.matmul(
                        ps,
                        lhsT=x_v[hh][:, kc, col : col + 128],
                        rhs=wv_sb[:, kc, :],
                        start=(kc == 0),
                        stop=(kc == NKC - 1),
                    )
                psr = ps.rearrange("p (h d) -> p h d", h=HPC)
                if "v" in bias_tiles:
                    nc.vector.tensor_add(
                        out=Vaug[:, lkt, :, 0:64],
                        in0=psr,
                        in1=bias_tiles["v"].rearrange("p (h d) -> p h d", h=HPC),
                    )
                else:
                    nc.scalar.copy(out=Vaug[:, lkt, :, 0:64], in_=psr)

            for lqh in range(2):
                for mt in range(2):
                    emit_qk_proj("k", x_k, wk_sb, KT, mt, lqh)
            for lqh in range(2):
                for mt in range(2):
                    emit_qk_proj("q", x_q, wq_sb, QT, mt, lqh)
            for lkt in range(8):
                emit_v_lkt(lkt)

            # ---- global software-pipelined attention tick stream ----
            o_tiles = {}  # block -> {hp: psum tile}
            tmp_tiles = {}  # pair index -> tmp2 tile
            p_tiles = {}  # pair index -> p2 tile
            mb_tiles = {}  # g -> mask tile
            nti = 0  # global hp-tile counter for offload pattern

            def front(g):
                nonlocal nti
                blk, k = g // NLKT, g % NLKT
                pr, half = BLOCKS[blk]
                j = g // 2
                g_sb = gp.tile([128, 2 * HF], BF16, tag="g")
                nc.sync.dma_start(out=g_sb, in_=gPK[pr, half, k, :, :])
                if g % 2 == 0:
                    tmp_tiles[j] = tp.tile(
                        [128, 2, 2 * HF], BF16, tag="tmp", name=f"tmp{j}"
                    )
                tmp2 = tmp_tiles[j]
                for hp in range(2):
                    s_w = ps_s.tile([128, HF], F32, tag="s", name=f"s_{g}_{hp}")
                    for c in range(2):
                        nc.tensor.matmul(
                            s_w[:, c * 512 : (c + 1) * 512],
                            lhsT=KT[
                                hp * 64 : hp * 64 + 64,
                                pr,
                                k * 128 : (k + 1) * 128,
                            ],
                            rhs=QT[
                                hp * 64 : hp * 64 + 64,
                                pr,
                                half * HF + c * 512 : half * HF + (c + 1) * 512,
                            ],
                            start=True,
                            stop=True,
                        )
                    dst = tmp2[:, g % 2, hp * HF : (hp + 1) * HF]
                    nti += 1
                    if OFFLOAD_EVERY and nti % OFFLOAD_EVERY == 0:
                        sB = sb8.tile([128, HF], BF16, tag="sb")
                        nc.scalar.copy(out=sB, in_=s_w)
                        nc.gpsimd.tensor_mul(
                            dst, sB, g_sb[:, hp * HF : (hp + 1) * HF]
                        )
                    else:
                        nc.vector.tensor_mul(
                            out=dst,
                            in0=s_w,
                            in1=g_sb[:, hp * HF : (hp + 1) * HF],
                        )
                if g % 2 == 1:
                    p2 = pp.tile([128, 2, 2 * HF], BF16, tag="p")
                    nc.scalar.activation(
                        out=p2, in_=tmp_tiles.pop(j), func=AF.Exp,
                        bias=ebias, scale=1.0,
                    )
                    p_tiles[j] = p2
                    if masked:
                        for par in range(2):
                            gg = 2 * j + par
                            kk = gg % NLKT
                            _, hf2 = BLOCKS[gg // NLKT]
                            mb_sb = gp.tile([128, HF], BF16, tag="mb")
                            nc.sync.dma_start(
                                out=mb_sb,
                                in_=mbT[
                                    kk * 128 : (kk + 1) * 128,
                                    hf2 * HF : (hf2 + 1) * HF,
                                ],
                            )
                            for hp in range(2):
                                nc.vector.tensor_mul(
                                    out=p2[:, par, hp * HF : (hp + 1) * HF],
                                    in0=p2[:, par, hp * HF : (hp + 1) * HF],
                                    in1=mb_sb,
                                )

            def back(g):
                blk, k = g // NLKT, g % NLKT
                pr, half = BLOCKS[blk]
                if k == 0:
                    o_tiles[blk] = {
                        hp: ps_o.tile(
                            [128, HF], F32, tag="o", name=f"o_{blk}_{hp}"
                        )
                        for hp in range(2)
                    }
                j = g // 2
                p2 = p_tiles[j]
                for hp in range(2):
                    for c in range(2):
                        nc.tensor.matmul(
                            o_tiles[blk][hp][:, c * 512 : (c + 1) * 512],
                            lhsT=Vaug[:, k, 2 * pr + hp, :],
                            rhs=p2[
                                :, g % 2, hp * HF + c * 512 : hp * HF + (c + 1) * 512
                            ],
                            start=(k == 0),
                            stop=(k == NLKT - 1),
                        )
                if g % 2 == 1:
                    p_tiles.pop(j)
                if k == NLKT - 1:
                    OUa = opl.tile([65, 2, HF], BF16, tag="ou")
                    for hp in range(2):
                        nc.scalar.copy(
                            out=OUa[:, hp, :], in_=o_tiles[blk][hp][0:65, :]
                        )
                    nc.gpsimd.dma_start(out=oU[pr, half, :, :, :], in_=OUa)

            # second half of the v projections rides the early ticks; all of
            # them must release the "o" psum ring before back(0) claims it.
            VSCHED = {0: [8], 1: [9], 2: [10], 3: [11], 4: [12, 13], 5: [14, 15]}
            assert max(VSCHED) < PIPE
            NG = 4 * NLKT
            for g in range(NG + PIPE):
                if g < NG:
                    front(g)
                for lkt in VSCHED.get(g, ()):
                    emit_v_lkt(lkt)
                if g >= PIPE:
                    back(g - PIPE)

    nc.finalize()
    return nc


def build_l2(use_bo: bool, use_gamma: bool, use_beta: bool):
    nc = bacc.Bacc("TRN2", target_bir_lowering=False)

    oTf = nc.declare_dram_parameter("oTf", [128, NKC, CH], BF16, isOutput=False)
    rdK = nc.declare_dram_parameter("rdK", [128, NKC, CH], BF16, isOutput=False)
    woTs = nc.declare_dram_parameter("woTs", [128, NKC, D], BF16, isOutput=False)
    qres = nc.declare_dram_parameter("qres", [4, 128, D], BF16, isOutput=False)
    if use_bo:
        boR = nc.declare_dram_parameter("boR", [1, D], F32, isOutput=False)
    if use_gamma:
        gaR = nc.declare_dram_parameter("gaR", [1, D], F32, isOutput=False)
    if use_beta:
        beR = nc.declare_dram_parameter("beR", [1, D], F32, isOutput=False)
    yout = nc.declare_dram_parameter("yout", [4, 128, D], BF16, isOutput=True)

    with tile.TileContext(nc) as tc:
        with (
            tc.tile_pool(name="ins", bufs=1) as ins,
            tc.tile_pool(name="res", bufs=4) as res,
            tc.tile_pool(name="xb", bufs=4) as xb,
            tc.tile_pool(name="st", bufs=4) as st,
            tc.tile_pool(name="ps", bufs=8, space="PSUM") as psp,
        ):
            oT_sb = ins.tile([128, NKC, CH], BF16, tag="ot")
            rd_sb = ins.tile([128, NKC, CH], BF16, tag="rd")
            wo_sb = ins.tile([128, NKC, D], BF16, tag="wo")
            oTn = ins.tile([128, NKC, CH], BF16, tag="on")
            for kc in range(NKC):
                nc.sync.dma_start(out=oT_sb[:, kc, :], in_=oTf[:, kc, :])
                nc.sync.dma_start(out=rd_sb[:, kc, :], in_=rdK[:, kc, :])
                nc.sync.dma_start(out=wo_sb[:, kc, :], in_=woTs[:, kc, :])
            eps_sb = ins.tile([128, 1], F32, tag="eps")
            nc.vector.memset(eps_sb, EPS)
            bo_sb = ga_sb = be_sb = None
            if use_bo:
                bo_sb = ins.tile([128, D], F32, tag="bo")
                nc.sync.dma_start(out=bo_sb, in_=boR.ap().to_broadcast([128, D]))
            if use_gamma:
                ga_sb = ins.tile([128, D], F32, tag="ga")
                nc.sync.dma_start(out=ga_sb, in_=gaR.ap().to_broadcast([128, D]))
            if use_beta:
                be_sb = ins.tile([128, D], F32, tag="be")
                nc.sync.dma_start(out=be_sb, in_=beR.ap().to_broadcast([128, D]))

            q_tiles = []
            for m in range(4):
                q_sb = res.tile([128, D], BF16, tag="q", name=f"q{m}")
                nc.gpsimd.dma_start(out=q_sb, in_=qres[m, :, :])
                q_tiles.append(q_sb)

            # warm-up matmuls
            warm = psp.tile([128, 512], F32, tag="mm", name="warm")
            for i in range(8):
                nc.tensor.matmul(
                    warm,
                    lhsT=wo_sb[:, 0, 0:128],
                    rhs=wo_sb[:, 0, 0:512],
                    start=True,
                    stop=True,
                    skip_group_check=True,
                )

            # normalize per kc (unlocks matmuls as DMA lands)
            for kc in range(NKC):
                nc.vector.tensor_mul(
                    out=oTn[:, kc, :], in0=oT_sb[:, kc, :], in1=rd_sb[:, kc, :]
                )

            fused_ln = bo_sb is None
            ps_mn = {
                (m, n): psp.tile([128, 512], F32, tag="mm", name=f"mm{m}{n}")
                for m in range(4)
                for n in range(2)
            }
            for kc in range(NKC):
                for m in range(4):
                    for n in range(2):
                        nc.tensor.matmul(
                            ps_mn[(m, n)],
                            lhsT=oTn[:, kc, m * 128 : (m + 1) * 128],
                            rhs=wo_sb[:, kc, n * 512 : (n + 1) * 512],
                            start=(kc == 0),
                            stop=(kc == NKC - 1),
                        )

            for m in range(4):
                q_sb = q_tiles[m]
                x = xb.tile([128, D], F32, tag="x")
                accs = st.tile([128, 2], F32, tag="accs")
                for n in range(2):
                    ps = ps_mn.pop((m, n))
                    if fused_ln:
                        nc.vector.scalar_tensor_tensor(
                            out=x[:, n * 512 : (n + 1) * 512],
                            in0=ps,
                            scalar=1.0,
                            in1=q_sb[:, n * 512 : (n + 1) * 512],
                            op0=mybir.AluOpType.mult,
                            op1=mybir.AluOpType.add,
                            accum_out=accs[:, n : n + 1],
                        )
                    else:
                        nc.vector.tensor_add(
                            out=x[:, n * 512 : (n + 1) * 512],
                            in0=ps,
                            in1=q_sb[:, n * 512 : (n + 1) * 512],
                        )
                if fused_ln:
                    scr = xb.tile([128, D], F32, tag="scr")
                    ssq = st.tile([128, 1], F32, tag="ssq")
                    nc.scalar.activation(
                        out=scr, in_=x, func=AF.Square, accum_out=ssq
                    )
                    mu = st.tile([128, 1], F32, tag="mu")
                    nc.vector.tensor_scalar(
                        out=mu,
                        in0=accs[:, 0:1],
                        scalar1=accs[:, 1:2],
                        scalar2=1.0 / D,
                        op0=mybir.AluOpType.add,
                        op1=mybir.AluOpType.mult,
                    )
                    musq = st.tile([128, 1], F32, tag="musq")
                    nc.vector.tensor_mul(out=musq, in0=mu, in1=mu)
                    var = st.tile([128, 1], F32, tag="var")
                    nc.vector.tensor_scalar(
                        out=var,
                        in0=ssq,
                        scalar1=1.0 / D,
                        scalar2=musq,
                        op0=mybir.AluOpType.mult,
                        op1=mybir.AluOpType.subtract,
                    )
                    std = st.tile([128, 1], F32, tag="std")
                    nc.scalar.activation(
                        out=std, in_=var, func=AF.Sqrt, bias=eps_sb, scale=1.0
                    )
                else:
                    if bo_sb is not None:
                        nc.vector.tensor_add(out=x, in0=x, in1=bo_sb)
                    stats = st.tile([128, 2, 6], F32, tag="stats")
                    for hh in range(2):
                        nc.vector.bn_stats(
                            out=stats[:, hh, :],
                            in_=x[:, hh * 512 : (hh + 1) * 512],
                        )
                    mv = st.tile([128, 2], F32, tag="mv")
                    nc.vector.bn_aggr(out=mv, in_=stats)
                    mu = mv[:, 0:1]
                    std = st.tile([128, 1], F32, tag="std")
                    nc.scalar.activation(
                        out=std, in_=mv[:, 1:2], func=AF.Sqrt, bias=eps_sb, scale=1.0
                    )
                rstd = st.tile([128, 1], F32, tag="rstd")
                nc.vector.reciprocal(out=rstd, in_=std)
                y = xb.tile([128, D], BF16, tag="y")
                nc.vector.tensor_scalar(
                    out=y,
                    in0=x,
                    scalar1=mu,
                    scalar2=rstd,
                    op0=mybir.AluOpType.subtract,
                    op1=mybir.AluOpType.mult,
                )
                if ga_sb is not None:
                    nc.vector.tensor_mul(out=y, in0=y, in1=ga_sb)
                if be_sb is not None:
                    nc.vector.tensor_add(out=y, in0=y, in1=be_sb)
                nc.gpsimd.dma_start(out=yout[m, :, :], in_=y)

    nc.finalize()
    return nc


_L1_CACHE = {}
_L2_CACHE = {}
LAST_RUNS = []  # (tag, nc, in_maps) of the most recent kernel() call, for profiling


def kernel(
    q, k, v, k_gate, mask, wq, bq, wk, bk, wv, bv, wo, bo, gamma, beta
):
    q = np.asarray(q, np.float32)
    k = np.asarray(k, np.float32)
    v = np.asarray(v, np.float32)
    k_gate = np.asarray(k_gate, np.float32)
    mask = np.asarray(mask)
    wq = np.asarray(wq, np.float32)
    wk = np.asarray(wk, np.float32)
    wv = np.asarray(wv, np.float32)
    wo = np.asarray(wo, np.float32)
    bq = np.asarray(bq, np.float32)
    bk = np.asarray(bk, np.float32)
    bv = np.asarray(bv, np.float32)
    bo = np.asarray(bo, np.float32)
    gamma = np.asarray(gamma, np.float32)
    beta = np.asarray(beta, np.float32)

    masked = bool(mask.any())
    use_bq = bool(np.any(bq))
    use_bk = bool(np.any(bk))
    use_bv = bool(np.any(bv))
    use_bo = bool(np.any(bo))
    use_gamma = bool(np.any(gamma != 1.0))
    use_beta = bool(np.any(beta))

    temp = float(np.float32(np.power(DK, 0.5)))

    key1 = (masked, use_bq, use_bk, use_bv)
    if key1 not in _L1_CACHE:
        _L1_CACHE[key1] = build_l1(*key1)
    nc1 = _L1_CACHE[key1]

    # ---- stage launch-1 inputs ----
    xT = {}  # (name, b) -> [128, NKC, L] bf16
    for b in range(B):
        xT[("q", b)] = _bf(_kc_layout(q[b].T))
        xT[("k", b)] = _bf(_kc_layout(k[b].T))
        xT[("v", b)] = _bf(_kc_layout(v[b].T))
    wts = {}  # (name, hg) -> [128, NKC, MPC] bf16
    for hg in range(4):
        sl = slice(hg * MPC, (hg + 1) * MPC)
        wts[("q", hg)] = _bf(_kc_layout(wq[sl].T / temp))
        wts[("k", hg)] = _bf(_kc_layout(wk[sl].T))
        wts[("v", hg)] = _bf(_kc_layout(wv[sl].T))

    in_maps = []
    for c in range(NCORE):
        b, hg = c // 4, c % 4
        hsl = slice(hg * HPC, (hg + 1) * HPC)
        # gate pack: k_gate[b] is [head, lq, lk]; we need the transposed
        # per-tile layout gPK[pr, half, lkt, p, hp*1024 + i] = g[h, lq, lk]
        gh = k_gate[b, hsl]  # [4, 2048, 2048]  (head, lq, lk)
        gr = gh.reshape(2, 2, 2, HF, NLKT, 128)  # pr, hp, half, i, lkt, p
        gPK = _bf(gr.transpose(0, 2, 4, 5, 1, 3).reshape(2, 2, NLKT, 128, 2 * HF))
        m = {
            "qT": xT[("q", b)],
            "kT": xT[("k", b)],
            "vT": xT[("v", b)],
            "wqT": wts[("q", hg)],
            "wkT": wts[("k", hg)],
            "wvT": wts[("v", hg)],
            "gPK": gPK,
        }
        if use_bq:
            m["bqP"] = np.ascontiguousarray(
                (bq[hg * MPC : (hg + 1) * MPC] / temp).reshape(2, 128).T
            )
        if use_bk:
            m["bkP"] = np.ascontiguousarray(
                bk[hg * MPC : (hg + 1) * MPC].reshape(2, 128).T
            )
        if use_bv:
            m["bvR"] = bv[hg * MPC : (hg + 1) * MPC].reshape(1, MPC).copy()
        if masked:
            m["mbT"] = _bf((~mask[b]).astype(np.float32).T)
        in_maps.append(m)

    LAST_RUNS.clear()
    LAST_RUNS.append(("L1", nc1, in_maps))
    res1 = run_bass_kernel_spmd(nc1, in_maps, list(range(NCORE)))

    # assemble O_un^T per batch [1024, L] + denominators
    OTb = np.empty((B, H * DV, L), np.float32)
    DENb = np.empty((B, H, L), np.float32)
    for b in range(B):
        for hg in range(4):
            r = res1.results[b * 4 + hg]["oU"].astype(np.float32)
            # r: [pr, half, 65, hp, HF]
            for pr in range(2):
                for hp in range(2):
                    h = hg * 4 + 2 * pr + hp
                    blk = r[pr, :, :, hp, :]  # [half, 65, HF]
                    OTb[b, h * 64 : (h + 1) * 64, :] = np.concatenate(
                        [blk[0, :64], blk[1, :64]], axis=1
                    )
                    DENb[b, h, :HF] = blk[0, 64]
                    DENb[b, h, HF:] = blk[1, 64]

    key2 = (use_bo, use_gamma, use_beta)
    if key2 not in _L2_CACHE:
        _L2_CACHE[key2] = build_l2(*key2)
    nc2 = _L2_CACHE[key2]

    woTs = _bf(_kc_layout(wo.T))
    rd_full = 1.0 / DENb  # [B, H, L]
    in_maps2 = []
    for c in range(NCORE):
        b, rchunk = c // 4, c % 4
        rows = slice(rchunk * CH, (rchunk + 1) * CH)
        otf = OTb[b][:, rows]  # [1024, 512]
        # rdK[p, kc, i] = 1/den[head = 2*kc + p//64, row_i]
        rdc = rd_full[b][:, rows]  # [16, 512]
        rdK = np.empty((128, NKC, CH), np.float32)
        for kc in range(NKC):
            rdK[0:64, kc, :] = rdc[2 * kc]
            rdK[64:128, kc, :] = rdc[2 * kc + 1]
        m = {
            "oTf": _bf(otf.reshape(NKC, 128, CH).transpose(1, 0, 2)),
            "rdK": _bf(rdK),
            "woTs": woTs,
            "qres": _bf(q[b, rows].reshape(4, 128, D)),
        }
        if use_bo:
            m["boR"] = bo.reshape(1, D).copy()
        if use_gamma:
            m["gaR"] = gamma.reshape(1, D).copy()
        if use_beta:
            m["beR"] = beta.reshape(1, D).copy()
        in_maps2.append(m)

    LAST_RUNS.append(("L2", nc2, in_maps2))
    res2 = run_bass_kernel_spmd(nc2, in_maps2, list(range(NCORE)))

    out = np.empty((B, L, D), np.float32)
    for c in range(NCORE):
        b, rchunk = c // 4, c % 4
        out[b, rchunk * CH : (rchunk + 1) * CH] = (
            res2.results[c]["yout"].astype(np.float32).reshape(CH, D)
        )
    return out
